# revision 20
# baseline (speedup 1.0000x reference)
"""Trainium2 Bass kernel for nn_AdaptiveContactZones.

Computes, for each batch: pairwise distances hand[778,3] x obj[8192,3],
min over obj (KNN k=1), sqrt, proximity mask, and top-50 closest hand
vertices (indices + distances, jax.lax.top_k semantics incl. stable
tie-breaking).

The kernel replicates the reference's float arithmetic exactly as it
executes on the neuron backend:
    cross = fp32 PE matmul, hand stationary       (bit-exact vs XLA einsum)
    d2    = fl(fl(h2 + o2) - fl(2*cross))         (tree A, fused via
                                                   scalar_tensor_tensor)
    min_d2 = min over obj axis, clamped at 0
    min_dists = ACT Sqrt spline (same as XLA's lowering)
    top-50 = stable rank selection (rank = #less + #equal-before)

Sharding: data-parallel over batch, 4 batches per NeuronCore, 8 cores.
"""

import numpy as np

import concourse.bacc as bacc
import concourse.mybir as mybir
from concourse.bass_utils import run_bass_kernel_spmd
from concourse import tile

F32 = mybir.dt.float32
AF = mybir.ActivationFunctionType
OP = mybir.AluOpType

B, H, V = 32, 778, 8192
NCORES, BPC = 8, 4          # cores, batches per core
HP, MT = 896, 7             # padded H = 7*128, h-tile count
KSEL = 50
PROX = 0.015
PAD_COORD = 1000.0          # padding hand verts -> huge distance, excluded
NCHUNK = 512                # matmul moving dim (1 PSUM bank fp32)
SPAN = 2048                 # DVE scan span (4 banks)
NSPAN = V // SPAN           # 4 spans per (batch, h-tile)
REP = 4                     # row-group replicas for 4-way PE concurrency
OUTW = HP + HP + 2 * KSEL   # per-batch output floats: md, mask, (dist,idx)*50

_CACHE = {}


def _build(scan_path="stt", rank_ltc="reduce", tie_path="tri"):
    nc = bacc.Bacc("TRN2", target_bir_lowering=False, debug=False)

    # ---- dram I/O (per core) ----
    hand_d = nc.dram_tensor("hand", [BPC, HP, 3], F32, kind="ExternalInput").ap()
    handT_d = nc.dram_tensor("handT", [BPC, 3, HP], F32, kind="ExternalInput").ap()
    obj_d = nc.dram_tensor("obj", [BPC, V, 3], F32, kind="ExternalInput").ap()
    objT_d = nc.dram_tensor("objT", [BPC, 3, V], F32, kind="ExternalInput").ap()
    eye_d = nc.dram_tensor("eye", [128, 128], F32, kind="ExternalInput").ap()
    iota50_d = nc.dram_tensor("iota50", [128, KSEL], F32, kind="ExternalInput").ap()
    hidx_d = nc.dram_tensor("hidx", [128, MT], F32, kind="ExternalInput").ap()
    tri_d = nc.dram_tensor("tri", [128, 128], F32, kind="ExternalInput").ap()
    out_d = nc.dram_tensor("out", [BPC, OUTW], F32, kind="ExternalOutput").ap()
    o2row_d = nc.dram_tensor("o2row_b", [BPC, V], F32).ap()
    mdrow_d = nc.dram_tensor("mdrow_b", [BPC, HP], F32).ap()

    with tile.TileContext(nc) as tc:
        with (
            tc.tile_pool(name="const", bufs=1) as cpool,
            tc.tile_pool(name="rhs", bufs=2) as rhspool,
            tc.tile_pool(name="lhs", bufs=2) as lhspool,
            tc.tile_pool(name="prep", bufs=2) as prep,
            tc.tile_pool(name="scan", bufs=2) as scan,
            tc.tile_pool(name="epi", bufs=2) as epi,
            tc.tile_pool(name="epibig", bufs=1) as epibig,
            tc.tile_pool(name="ps", bufs=2, space="PSUM") as ps,
        ):
            # ---- constants ----
            eye = cpool.tile([128, 128], F32, tag="eye")
            nc.sync.dma_start(eye[:], eye_d[:])
            iota50 = cpool.tile([128, KSEL], F32, tag="iota50")
            nc.sync.dma_start(iota50[:], iota50_d[:])
            hidx = cpool.tile([128, MT], F32, tag="hidx")
            nc.sync.dma_start(hidx[:], hidx_d[:])
            tri = cpool.tile([128, 128], F32, tag="tri")
            nc.sync.dma_start(tri[:], tri_d[:])

            for g in range(BPC):
                # ============ prep for batch g ============
                # hand in (p, m) layout: [128, 7, 3], h = 128m + p
                hand_pm = prep.tile([128, MT, 3], F32, tag="hand_pm")
                nc.sync.dma_start(
                    hand_pm[:], hand_d[g].rearrange("(m p) c -> p m c", p=128)
                )
                # h2 in (p, m) layout, exact tree (x*x + y*y) + z*z
                h2pm = prep.tile([128, MT], F32, tag="h2pm")
                t_a = prep.tile([128, MT], F32, tag="t_a")
                nc.vector.tensor_mul(t_a[:], hand_pm[:, :, 0], hand_pm[:, :, 0])
                nc.vector.tensor_mul(h2pm[:], hand_pm[:, :, 1], hand_pm[:, :, 1])
                nc.vector.tensor_add(h2pm[:], t_a[:], h2pm[:])
                nc.vector.tensor_mul(t_a[:], hand_pm[:, :, 2], hand_pm[:, :, 2])
                nc.vector.tensor_add(h2pm[:], h2pm[:], t_a[:])

                # obj natural [128, 64, 3], v = 64p + j (identity order)
                obj_nat = prep.tile([128, 64, 3], F32, tag="obj_nat")
                nc.sync.dma_start(
                    obj_nat[:], obj_d[g].rearrange("(p j) c -> p j c", p=128)
                )
                o2nat = prep.tile([128, 64], F32, tag="o2nat")
                t_b = prep.tile([128, 64], F32, tag="t_b")
                nc.vector.tensor_mul(t_b[:], obj_nat[:, :, 0], obj_nat[:, :, 0])
                nc.vector.tensor_mul(o2nat[:], obj_nat[:, :, 1], obj_nat[:, :, 1])
                nc.vector.tensor_add(o2nat[:], t_b[:], o2nat[:])
                nc.vector.tensor_mul(t_b[:], obj_nat[:, :, 2], obj_nat[:, :, 2])
                nc.vector.tensor_add(o2nat[:], o2nat[:], t_b[:])

                # flatten o2 to a dram row (p-major = v order), then
                # broadcast to all partitions by log-doubling DMAs
                nc.sync.dma_start(
                    o2row_d[g].rearrange("(p j) -> p j", p=128), o2nat[:, :]
                )
                o2b = prep.tile([128, V], F32, tag="o2b")
                nc.sync.dma_start(
                    o2b[:, :], o2row_d[g : g + 1, :].broadcast_to((128, V))
                )

                # lhs for cross MM: +2 * handT, replicated in 4 row groups
                l2h = prep.tile([3, HP], F32, tag="l2h")
                nc.sync.dma_start(l2h[:], handT_d[g])
                nc.scalar.mul(l2h[:], l2h[:], 2.0)
                lhsC = lhspool.tile([128, HP], F32, tag="lhsC")
                for r in range(REP):
                    nc.sync.dma_start(lhsC[32 * r : 32 * r + 3, :], l2h[:])

                # rhs for cross MM: objT, replicated in 4 row groups
                rhsC = rhspool.tile([128, V], F32, tag="rhsC")
                for r in range(REP):
                    nc.sync.dma_start(rhsC[32 * r : 32 * r + 3, :], objT_d[g])

                # ============ scan: per h-tile ============
                md2 = epi.tile([128, MT], F32, tag="md2")
                for m in range(MT):
                    accs = scan.tile([128, NSPAN], F32, tag="accs")
                    for s in range(NSPAN):
                        ptile = ps.tile([128, SPAN], F32, tag="pscan")
                        for q in range(REP):
                            c0 = s * SPAN + q * NCHUNK
                            nc.tensor.matmul(
                                ptile[:, q * NCHUNK : (q + 1) * NCHUNK],
                                lhsC[32 * q : 32 * q + 3, 128 * m : 128 * (m + 1)],
                                rhsC[32 * q : 32 * q + 3, c0 : c0 + NCHUNK],
                                start=True,
                                stop=True,
                                tile_position=(32 * q, 0),
                            )
                        # d2 = fl(fl(o2 + h2) - 2c)  (exact reference tree)
                        d2t = scan.tile([128, SPAN], F32, tag="d2t")
                        nc.vector.scalar_tensor_tensor(
                            d2t[:],
                            o2b[:, s * SPAN : (s + 1) * SPAN],
                            h2pm[:, m : m + 1],
                            ptile[:],
                            op0=OP.add,
                            op1=OP.subtract,
                        )
                        nc.vector.tensor_reduce(
                            accs[:, s : s + 1], d2t[:], axis=mybir.AxisListType.X,
                            op=OP.min,
                        )
                    nc.vector.tensor_reduce(
                        md2[:, m : m + 1], accs[:], axis=mybir.AxisListType.X,
                        op=OP.min,
                    )

                # ============ epilogue for batch g ============
                # clamp and sqrt (ACT spline = reference bits)
                nc.vector.tensor_scalar_max(md2[:], md2[:], 0.0)
                md = epi.tile([128, MT], F32, tag="md")
                nc.scalar.activation(md[:], md2[:], AF.Sqrt)

                # transpose md -> [7, 128] (m on partitions)
                pT = ps.tile([128, SPAN], F32, tag="pscan")
                nc.tensor.transpose(pT[0:MT, 0:128], md[:, 0:MT], eye[:])
                mdT = epi.tile([MT, 128], F32, tag="mdT")
                nc.vector.tensor_copy(mdT[:], pT[0:MT, 0:128])

                # flatten to dram row, broadcast to all partitions
                nc.sync.dma_start(
                    mdrow_d[g].rearrange("(m p) -> m p", m=MT), mdT[:, :]
                )
                mdb = epibig.tile([128, HP], F32, tag="mdb")
                nc.sync.dma_start(
                    mdb[:, :], mdrow_d[g : g + 1, :].broadcast_to((128, HP))
                )

                # ranks: #less + #equal-before  (exact top_k tie semantics)
                rank = epi.tile([128, MT], F32, tag="rank")
                teq = epi.tile([128, MT], F32, tag="teq")
                trash = epibig.tile([128, HP], F32, tag="trash")
                eqt = epibig.tile([128, HP], F32, tag="eqt")
                teq2 = epi.tile([128, MT], F32, tag="teq2")
                for m in range(MT):
                    # #less count: explicit compare + reduce
                    nc.vector.tensor_scalar(
                        out=trash[:], in0=mdb[:], scalar1=md[:, m : m + 1],
                        scalar2=None, op0=OP.is_lt,
                    )
                    nc.vector.tensor_reduce(
                        rank[:, m : m + 1], trash[:],
                        axis=mybir.AxisListType.X, op=OP.add,
                    )
                    # equal-before count over j in [0, h)
                    nc.vector.tensor_scalar(
                        out=eqt[:], in0=mdb[:], scalar1=md[:, m : m + 1],
                        scalar2=None, op0=OP.is_equal,
                    )
                    if tie_path == "tmr":
                        nc.vector.tensor_mask_reduce(
                            trash[:], eqt[:], 0.0, hidx[:, m : m + 1], 1.0, 0.0,
                            op=OP.add, accum_out=teq[:, m : m + 1],
                        )
                    else:
                        # blocks before tile m: plain sum; diagonal block:
                        # strict-lower triangular mask (per-partition row)
                        nc.vector.tensor_mul(
                            trash[:, 0:128], eqt[:, 128 * m : 128 * (m + 1)],
                            tri[:],
                        )
                        nc.vector.tensor_reduce(
                            teq[:, m : m + 1], trash[:, 0:128],
                            axis=mybir.AxisListType.X, op=OP.add,
                        )
                        if m > 0:
                            nc.vector.tensor_reduce(
                                teq2[:, m : m + 1], eqt[:, 0 : 128 * m],
                                axis=mybir.AxisListType.X, op=OP.add,
                            )
                            nc.vector.tensor_add(
                                teq[:, m : m + 1], teq[:, m : m + 1],
                                teq2[:, m : m + 1],
                            )
                nc.vector.tensor_add(rank[:], rank[:], teq[:])

                # one-hot selection matmuls: sel[50, 2] = sum_m C_m^T @ (md, h)
                psel = ps.tile([128, SPAN], F32, tag="pscan")
                cm = epi.tile([128, KSEL], F32, tag="cm")
                payload = epi.tile([128, 2], F32, tag="payload")
                for m in range(MT):
                    nc.vector.tensor_scalar(
                        out=cm[:], in0=iota50[:], scalar1=rank[:, m : m + 1],
                        scalar2=None, op0=OP.is_equal,
                    )
                    nc.vector.tensor_copy(payload[:, 0:1], md[:, m : m + 1])
                    nc.vector.tensor_copy(payload[:, 1:2], hidx[:, m : m + 1])
                    nc.tensor.matmul(
                        psel[0:KSEL, 0:2], cm[:], payload[:],
                        start=(m == 0), stop=(m == MT - 1),
                    )
                sel = epi.tile([KSEL, 2], F32, tag="sel")
                nc.vector.tensor_copy(sel[:], psel[0:KSEL, 0:2])

                # proximity mask on mdT
                maskT = epi.tile([MT, 128], F32, tag="maskT")
                nc.vector.tensor_scalar(
                    out=maskT[:], in0=mdT[:], scalar1=PROX, scalar2=None,
                    op0=OP.is_lt,
                )

                # ---- outputs ----
                nc.sync.dma_start(
                    out_d[g, 0:HP].rearrange("(m p) -> m p", m=MT), mdT[:, :]
                )
                nc.sync.dma_start(
                    out_d[g, HP : 2 * HP].rearrange("(m p) -> m p", m=MT),
                    maskT[:, :],
                )
                nc.sync.dma_start(
                    out_d[g, 2 * HP : 2 * HP + 2 * KSEL].rearrange(
                        "(k two) -> k two", k=KSEL
                    ),
                    sel[:, :],
                )
    nc.compile()
    return nc


def _get_nc():
    if "nc" not in _CACHE:
        _CACHE["nc"] = _build()
    return _CACHE["nc"]


def _host_inputs(hand, obj):
    handp = np.full((B, HP, 3), PAD_COORD, np.float32)
    handp[:, :H] = hand
    eye = np.eye(128, dtype=np.float32)
    iota50 = np.broadcast_to(
        np.arange(KSEL, dtype=np.float32)[None, :], (128, KSEL)
    ).copy()
    hidx = (np.arange(MT, dtype=np.float32)[None, :] * 128.0
            + np.arange(128, dtype=np.float32)[:, None]).astype(np.float32)
    tri = (np.arange(128)[None, :] < np.arange(128)[:, None]).astype(np.float32)
    in_maps = []
    for c in range(NCORES):
        sl = slice(BPC * c, BPC * (c + 1))
        in_maps.append({
            "hand": np.ascontiguousarray(handp[sl]),
            "handT": np.ascontiguousarray(handp[sl].transpose(0, 2, 1)),
            "obj": np.ascontiguousarray(obj[sl]),
            "objT": np.ascontiguousarray(obj[sl].transpose(0, 2, 1)),
            "eye": eye,
            "iota50": iota50,
            "hidx": hidx,
            "tri": tri,
        })
    return in_maps


def kernel(hand_verts, obj_verts, iteration=None, _return_exec_time=False,
           _trace=False):
    hand = np.asarray(hand_verts, np.float32)
    obj = np.asarray(obj_verts, np.float32)
    assert hand.shape == (B, H, 3) and obj.shape == (B, V, 3)

    nc = _get_nc()
    in_maps = _host_inputs(hand, obj)
    kw = {}
    if _trace:
        kw = dict(trace=True)
    r = run_bass_kernel_spmd(nc, in_maps, list(range(NCORES)), **kw)

    outs = np.stack([r.results[c]["out"] for c in range(NCORES)])  # [8,4,OUTW]
    outs = outs.reshape(B, OUTW)
    min_dists = outs[:, :H].astype(np.float32)
    close_mask = outs[:, HP : HP + H] > 0.5
    sel = outs[:, 2 * HP : 2 * HP + 2 * KSEL].reshape(B, KSEL, 2)
    contact_dists = np.ascontiguousarray(sel[:, :, 0], dtype=np.float32)
    contact_indices = np.rint(sel[:, :, 1]).astype(np.int32)
    if _return_exec_time:
        return (min_dists, close_mask, contact_indices, contact_dists), r
    return min_dists, close_mask, contact_indices, contact_dists
